# revision 2
# baseline (speedup 1.0000x reference)
# Trainium2 Bass kernel for nn_DeformSpaceAttentionv2 — PE-matmul rewrite.
#
# Math: logits = A @ feat + c0 with A = W1*diag(gamma/sqrt(var+eps))*W0 (4x256),
# feat[c] = max_k bilinear_k(x)[c]. Per core (batch x 32-row band):
#  - bilinear corner weights + dma_gather indices are host-marshaled,
#  - TWO dma_gathers per (k, 16-row block): 1024 descriptors each from the
#    4-corner neighborhood table in HBM (T[s] = x fp8 at s, s+1, s+128, s+129),
#  - corner MAC on the TensorEngine: out[c, x] = sum_x' G[x', c] * diag(w)[x', x]
#    with diag(w) built by 4x-mode tensor_scalar (id * w per-partition scalar)
#    on DVE (some on ACT); 4 corners accumulate in PSUM f32,
#  - ACT copies PSUM->SBUF bf16, DVE maxes over the 9 kernel points (2x mode),
#  - A-contraction as a tiny PE matmul over channel partitions, sigmoid(+c0)
#    on ACT, store [4, 32, 128] f32; host tiles channels 64x.
import numpy as np
import ml_dtypes

import concourse.bass as bass
import concourse.tile as tile
from concourse import mybir
from concourse.bass_utils import run_bass_kernel_spmd

BN_EPS = 1e-5
B, C, H, W = 2, 256, 128, 128
G4 = 4
ROWS = 32            # output rows per core
NCORES = 8
NK = 9
BLKS = 2
BLKROWS = 16
NC_ = NK * ROWS      # 288 sample columns per core
NT = 16788           # table rows

f32 = mybir.dt.float32
bf16 = mybir.dt.bfloat16
i16 = mybir.dt.int16
fp8 = mybir.dt.float8e4
i32 = mybir.dt.int32

_prog_cache = {}


def _split_waits(nc, max_waits=1):
    """walrus codegen supports only 1 sem-wait per instruction; split extras
    onto preceding NoOps."""
    for bb in nc.m.functions[0].blocks:
        new_insts = []
        for ins in bb.instructions:
            si = ins.sync_info
            if si is not None and si.on_wait and len(si.on_wait) > max_waits:
                waits = list(si.on_wait)
                extra, keep = waits[:-max_waits], waits[-max_waits:]
                for i in range(0, len(extra), max_waits):
                    chunk = extra[i:i + max_waits]
                    nop = mybir.InstNoOp(name=f"{ins.name}-wsplit-{i}", ins=[], outs=[])
                    nop.engine = ins.engine
                    nop.sync_info = mybir.SyncInfo(on_wait=chunk, on_update=[])
                    new_insts.append(nop)
                si.on_wait = keep
            new_insts.append(ins)
        bb.instructions[:] = new_insts


def _lower_libraries(nc):
    """Raw-Bass equivalents of the Bacc passes that make extended GPSIMD
    instructions (dma_gather) encodable: insert library LOAD_LIBs, then
    populate .instr bytes for InstISA subclasses."""
    import bass_rust as _bass_rust
    from concourse.library_config import all_libraries, standard
    inst_type_to_lib_mask = {}
    for lib in all_libraries:
        for inst_type in lib.instructions:
            inst_type_to_lib_mask[inst_type] = (
                inst_type_to_lib_mask.get(inst_type, 0) | (1 << lib.index))
    _bass_rust.insert_library_loads(nc, inst_type_to_lib_mask,
                                    len(all_libraries), standard.index)
    from concourse.library_overlay import lower_extended_insts
    lower_extended_insts(nc)


def _build_program():
    nc = bass.Bass("TRN2", target_bir_lowering=False)

    xf = nc.declare_dram_parameter("xf", [NT, 4 * C], fp8, isOutput=False)
    wtp = nc.declare_dram_parameter("wtp", [128, 4 * NC_], f32, isOutput=False)
    i16p = nc.declare_dram_parameter("i16p", [128, NK, BLKS, BLKROWS, 8], i16,
                                     isOutput=False)
    idn = nc.declare_dram_parameter("idn", [128, 128], bf16, isOutput=False)
    a2 = nc.declare_dram_parameter("a2", [128, 2 * G4], bf16, isOutput=False)
    c0p = nc.declare_dram_parameter("c0p", [G4, 1], f32, isOutput=False)
    out = nc.declare_dram_parameter("out", [G4, ROWS, W], f32, isOutput=True)

    Alu = mybir.AluOpType
    ACTF = mybir.ActivationFunctionType

    with tile.TileContext(nc) as tc:
        with (
            tc.tile_pool(name="consts", bufs=1) as consts,
            tc.tile_pool(name="gpool", bufs=4) as gpool,
            tc.tile_pool(name="dpool", bufs=2) as dpool,
            tc.tile_pool(name="spool", bufs=4) as spool,
            tc.tile_pool(name="mpool", bufs=1) as mpool,
            tc.tile_pool(name="opool", bufs=2) as opool,
            tc.tile_pool(name="psum", bufs=2, space="PSUM") as psum_pool,
        ):
            # ---- const loads ----
            I16 = consts.tile([128, NK, BLKS, BLKROWS, 8], i16)
            nc.sync.dma_start(out=I16, in_=i16p[:, :, :, :, :])
            wts_sb = consts.tile([128, 4 * NC_], f32)
            nc.sync.dma_start(out=wts_sb, in_=wtp[:, :])
            idn_sb = consts.tile([128, 128], bf16)
            nc.sync.dma_start(out=idn_sb, in_=idn[:, :])
            a2_sb = consts.tile([128, 2 * G4], bf16)
            nc.sync.dma_start(out=a2_sb, in_=a2[:, :])
            c0_sb = consts.tile([G4, 1], f32)
            nc.sync.dma_start(out=c0_sb, in_=c0p[:, :])

            TT = nc.vector.tensor_tensor
            TS = nc.vector.tensor_scalar

            def wcol(corner, col):
                return wts_sb[:, corner * NC_ + col:corner * NC_ + col + 1]

            # ---- main loop ----
            m_tiles = {}
            for blk in range(BLKS):
                for half in range(2):
                    m_tiles[(blk, half)] = mpool.tile(
                        [128, BLKROWS, 128], bf16,
                        tag=f"m{blk}{half}", name=f"m{blk}{half}")

            for blk in range(BLKS):
                for k in range(NK):
                    Gq = gpool.tile([128, BLKROWS, 4 * C], fp8, tag="Gq", name="Gq")
                    for hb in range(2):
                        nc.gpsimd.dma_gather(
                            out_ap=Gq[:, hb * 8:(hb + 1) * 8, :], in_ap=xf[:, :],
                            idxs_ap=I16[:, k, blk, hb * 8:(hb + 1) * 8, :],
                            num_idxs=1024, num_idxs_reg=1024, elem_size=4 * C)
                    s_tiles = [spool.tile([128, BLKROWS, 128], bf16, tag=f"s{h}",
                                          name=f"s{h}") for h in range(2)] \
                        if k > 0 else None
                    # build 16 diag sets for this (k, blk); reused by both halves
                    dts = []
                    for y in range(BLKROWS):
                        col = k * ROWS + blk * BLKROWS + y
                        dt = dpool.tile([128, 4, 128], bf16,
                                        tag=f"d{y}", name=f"d{y}")
                        for corner in range(4):
                            if y == 10:
                                nc.scalar.activation(
                                    out=dt[:, corner, :], in_=idn_sb[:],
                                    func=ACTF.Copy, scale=wcol(corner, col))
                            else:
                                TS(out=dt[:, corner, :], in0=idn_sb[:],
                                   scalar1=wcol(corner, col),
                                   scalar2=None, op0=Alu.mult)
                        dts.append(dt)
                    for half in range(2):
                        for yh in range(2):
                            ps = psum_pool.tile([128, 8, 128], f32, tag="ps",
                                                name="ps")
                            for yy in range(8):
                                y = yh * 8 + yy
                                for corner in range(4):
                                    nc.tensor.matmul(
                                        ps[:, yy, :],
                                        Gq[:, y, corner * C + half * 128:
                                           corner * C + half * 128 + 128],
                                        dts[y][:, corner, :],
                                        start=(corner == 0), stop=(corner == 3))
                            dst = m_tiles[(blk, half)] if k == 0 else s_tiles[half]
                            nc.scalar.activation(
                                out=dst[:, yh * 8:(yh + 1) * 8, :],
                                in_=ps[:], func=ACTF.Copy)
                    if k > 0:
                        for half in range(2):
                            m = m_tiles[(blk, half)]
                            TT(out=m[:], in0=s_tiles[half][:], in1=m[:], op=Alu.max)

                # ---- tail: A-contraction + sigmoid + store ----
                for q in range(4):
                    lg = psum_pool.tile([G4, 512], f32, tag="lg", name="lg")
                    for half in range(2):
                        nc.tensor.matmul(
                            lg[:],
                            a2_sb[:, half * G4:(half + 1) * G4],
                            m_tiles[(blk, half)][:, q * 4:(q + 1) * 4, :],
                            start=(half == 0), stop=(half == 1))
                    att = opool.tile([G4, 512], f32, tag="att", name="att")
                    nc.scalar.activation(out=att, in_=lg[:], func=ACTF.Sigmoid,
                                         bias=c0_sb[:, 0:1])
                    nc.sync.dma_start(
                        out=out[:, blk * BLKROWS + q * 4: blk * BLKROWS + (q + 1) * 4, :],
                        in_=att[:].rearrange("g (y x) -> g y x", y=4))

    _split_waits(nc)
    _lower_libraries(nc)
    return nc


def _marshal(inputs):
    x = np.ascontiguousarray(inputs["x"], dtype=np.float32)
    offset = np.ascontiguousarray(inputs["offset"], dtype=np.float32)
    W0 = np.asarray(inputs["W0"], np.float32); b0 = np.asarray(inputs["b0"], np.float32)
    gamma = np.asarray(inputs["gamma"], np.float32); beta = np.asarray(inputs["beta"], np.float32)
    rm = np.asarray(inputs["run_mean"], np.float32); rv = np.asarray(inputs["run_var"], np.float32)
    W1 = np.asarray(inputs["W1"], np.float32); b1 = np.asarray(inputs["b1"], np.float32)

    inv = gamma / np.sqrt(rv + BN_EPS)
    A = (W1 * inv[None, :]) @ W0              # (4, 256)
    c0 = W1 @ (inv * (b0 - rm) + beta) + b1   # (4,)

    a2 = np.zeros((128, 2 * G4), np.float32)
    a2[:, 0:G4] = A[:, 0:128].T
    a2[:, G4:2 * G4] = A[:, 128:256].T
    a2 = a2.astype(ml_dtypes.bfloat16)
    c0p = c0.reshape(G4, 1).astype(np.float32)
    idn = np.eye(128, dtype=ml_dtypes.bfloat16)

    ky = np.repeat(np.arange(-1, 2), 3).astype(np.float32)   # k//3 - 1
    kx = np.tile(np.arange(-1, 2), 3).astype(np.float32)     # k%3 - 1

    xf_b = []
    for b in range(B):
        # F' = image rows -2..129 zero-padded; table row s holds the 4 corner
        # pixel vectors at positions s, s+1, s+W, s+W+1 of the padded image.
        Ff = np.zeros((132 * W + 2 + 130, C), ml_dtypes.float8_e4m3)
        Ff[2 * W + 1:2 * W + 1 + H * W] = x[b].transpose(1, 2, 0).reshape(H * W, C).astype(ml_dtypes.float8_e4m3)
        T = np.concatenate([Ff[0:NT], Ff[1:NT + 1], Ff[W:NT + W], Ff[W + 1:NT + W + 1]], axis=1)
        xf_b.append(np.ascontiguousarray(T))

    xgrid = np.arange(128, dtype=np.float32)
    in_maps = []
    for core in range(NCORES):
        b = core // 4
        r0 = (core % 4) * ROWS
        # off[x, m=(k,y), c] = offset[b, 2k+c, r0+y, x]
        off = offset[b].reshape(NK, 2, H, W)[:, :, r0:r0 + ROWS, :]
        off_px = off.transpose(3, 0, 2, 1).reshape(128, NC_, 2)
        yv = np.arange(r0, r0 + ROWS, dtype=np.float32)
        ykc = np.broadcast_to((yv[None, :] + ky[:, None]).reshape(1, NC_), (128, NC_))
        xkc = (xgrid[:, None] + np.broadcast_to(
            kx[:, None], (NK, ROWS)).reshape(1, NC_))

        py = off_px[:, :, 0] + ykc                               # [x, (k,y)]
        px = off_px[:, :, 1] + xkc
        y0 = np.floor(py); x0 = np.floor(px)
        fy = (py - y0).astype(np.float32); fx = (px - x0).astype(np.float32)
        v0 = (y0 >= 0) & (y0 <= H - 1); v1 = (y0 + 1 >= 0) & (y0 + 1 <= H - 1)
        u0 = (x0 >= 0) & (x0 <= W - 1); u1 = (x0 + 1 >= 0) & (x0 + 1 <= W - 1)
        wy0 = (1.0 - fy) * v0; wy1 = fy * v1
        wx0 = (1.0 - fx) * u0; wx1 = fx * u1
        wtp = np.concatenate(
            [wy0 * wx0, wy0 * wx1, wy1 * wx0, wy1 * wx1], axis=1
        ).astype(np.float32)                                     # [128, 4*288]

        # gather indices in dma_gather's layout:
        # I16[16g + q, k, blk, y, xh] = sidx[x=16*xh+q, (k, blk*16+y)]
        sidx = (np.clip(y0, -2, 128) * 128 +
                np.clip(x0, -1, 128) + 257).astype(np.int16)
        s5 = sidx.reshape(8, 16, NK, BLKS, BLKROWS)              # [xh, q, k, b, y]
        i16c = np.tile(s5.transpose(1, 2, 3, 4, 0), (8, 1, 1, 1, 1)).copy()
        in_maps.append(dict(xf=xf_b[b], wtp=wtp, i16p=i16c,
                            idn=idn, a2=a2, c0p=c0p))
    return in_maps


def kernel(**inputs):
    if "nc" not in _prog_cache:
        _prog_cache["nc"] = _build_program()
    nc = _prog_cache["nc"]
    in_maps = _marshal(inputs)
    res = run_bass_kernel_spmd(nc, in_maps, list(range(NCORES)))
    out = np.zeros((B, C, H, W), np.float32)
    for core in range(NCORES):
        b = core // 4
        r0 = (core % 4) * ROWS
        att = res.results[core]["out"]               # (4, 32, 128)
        out[b, :, r0:r0 + ROWS, :] = np.tile(att, (C // G4, 1, 1))
    return out


# revision 4
# speedup vs baseline: 1.0305x; 1.0305x over previous
# Trainium2 Bass kernel for nn_DeformSpaceAttentionv2 — PE-matmul rewrite.
#
# Math: logits = A @ feat + c0 with A = W1*diag(gamma/sqrt(var+eps))*W0 (4x256),
# feat[c] = max_k bilinear_k(x)[c]. Per core (batch x 32-row band):
#  - bilinear corner weights + dma_gather indices are host-marshaled,
#  - TWO dma_gathers per (k, 16-row block): 1024 descriptors each from the
#    4-corner neighborhood table in HBM (T[s] = x fp8 at s, s+1, s+128, s+129),
#  - corner MAC on the TensorEngine: out[c, x] = sum_x' G[x', c] * diag(w)[x', x]
#    with diag(w) built by 4x-mode tensor_scalar (id * w per-partition scalar)
#    on DVE (some on ACT); 4 corners accumulate in PSUM f32,
#  - ACT copies PSUM->SBUF bf16, DVE maxes over the 9 kernel points (2x mode),
#  - A-contraction as a tiny PE matmul over channel partitions, sigmoid(+c0)
#    on ACT, store [4, 32, 128] f32; host tiles channels 64x.
import numpy as np
import ml_dtypes

import concourse.bass as bass
import concourse.tile as tile
from concourse import mybir
from concourse.bass_utils import run_bass_kernel_spmd

BN_EPS = 1e-5
B, C, H, W = 2, 256, 128, 128
G4 = 4
ROWS = 32            # output rows per core
NCORES = 8
NK = 9
BLKS = 2
BLKROWS = 16
NC_ = NK * ROWS      # 288 sample columns per core
NT = 16788           # table rows

f32 = mybir.dt.float32
bf16 = mybir.dt.bfloat16
i16 = mybir.dt.int16
fp8 = mybir.dt.float8e4
i32 = mybir.dt.int32

_prog_cache = {}


def _split_waits(nc, max_waits=1):
    """walrus codegen supports only 1 sem-wait per instruction; split extras
    onto preceding NoOps."""
    for bb in nc.m.functions[0].blocks:
        new_insts = []
        for ins in bb.instructions:
            si = ins.sync_info
            if si is not None and si.on_wait and len(si.on_wait) > max_waits:
                waits = list(si.on_wait)
                extra, keep = waits[:-max_waits], waits[-max_waits:]
                for i in range(0, len(extra), max_waits):
                    chunk = extra[i:i + max_waits]
                    nop = mybir.InstNoOp(name=f"{ins.name}-wsplit-{i}", ins=[], outs=[])
                    nop.engine = ins.engine
                    nop.sync_info = mybir.SyncInfo(on_wait=chunk, on_update=[])
                    new_insts.append(nop)
                si.on_wait = keep
            new_insts.append(ins)
        bb.instructions[:] = new_insts


def _lower_libraries(nc):
    """Raw-Bass equivalents of the Bacc passes that make extended GPSIMD
    instructions (dma_gather) encodable: insert library LOAD_LIBs, then
    populate .instr bytes for InstISA subclasses."""
    import bass_rust as _bass_rust
    from concourse.library_config import all_libraries, standard
    inst_type_to_lib_mask = {}
    for lib in all_libraries:
        for inst_type in lib.instructions:
            inst_type_to_lib_mask[inst_type] = (
                inst_type_to_lib_mask.get(inst_type, 0) | (1 << lib.index))
    _bass_rust.insert_library_loads(nc, inst_type_to_lib_mask,
                                    len(all_libraries), standard.index)
    from concourse.library_overlay import lower_extended_insts
    lower_extended_insts(nc)


def _build_program():
    nc = bass.Bass("TRN2", target_bir_lowering=False)

    xf = nc.declare_dram_parameter("xf", [NT, 4 * C], fp8, isOutput=False)
    wtp = nc.declare_dram_parameter("wtp", [128, 4 * NC_], f32, isOutput=False)
    i16p = nc.declare_dram_parameter("i16p", [128, NK, BLKS, BLKROWS, 8], i16,
                                     isOutput=False)
    idn = nc.declare_dram_parameter("idn", [128, 128], bf16, isOutput=False)
    a2 = nc.declare_dram_parameter("a2", [128, 2 * G4], bf16, isOutput=False)
    c0p = nc.declare_dram_parameter("c0p", [G4, 1], f32, isOutput=False)
    out = nc.declare_dram_parameter("out", [G4, ROWS, W], f32, isOutput=True)

    Alu = mybir.AluOpType
    ACTF = mybir.ActivationFunctionType

    with tile.TileContext(nc) as tc:
        with (
            tc.tile_pool(name="consts", bufs=1) as consts,
            tc.tile_pool(name="gpool", bufs=4) as gpool,
            tc.tile_pool(name="dpool", bufs=2) as dpool,
            tc.tile_pool(name="spool", bufs=4) as spool,
            tc.tile_pool(name="mpool", bufs=1) as mpool,
            tc.tile_pool(name="opool", bufs=3) as opool,
            tc.tile_pool(name="psum", bufs=3, space="PSUM") as psum_pool,
            tc.tile_pool(name="psumlg", bufs=2, space="PSUM") as lg_pool,
        ):
            # ---- const loads ----
            I16 = consts.tile([128, NK, BLKS, BLKROWS, 8], i16)
            nc.sync.dma_start(out=I16[:, 0:1, :, :, :], in_=i16p[:, 0:1, :, :, :])
            nc.sync.dma_start(out=I16[:, 1:NK, :, :, :], in_=i16p[:, 1:NK, :, :, :])
            wts_sb = consts.tile([128, 4 * NC_], f32)
            nc.sync.dma_start(out=wts_sb, in_=wtp[:, :])
            idn_sb = consts.tile([128, 128], bf16)
            nc.sync.dma_start(out=idn_sb, in_=idn[:, :])
            a2_sb = consts.tile([128, 2 * G4], bf16)
            nc.sync.dma_start(out=a2_sb, in_=a2[:, :])
            c0_sb = consts.tile([G4, 1], f32)
            nc.sync.dma_start(out=c0_sb, in_=c0p[:, :])

            TT = nc.vector.tensor_tensor
            TS = nc.vector.tensor_scalar

            def wcol(corner, col):
                return wts_sb[:, corner * NC_ + col:corner * NC_ + col + 1]

            # ---- main loop ----
            m_tiles = {}
            for blk in range(BLKS):
                for half in range(2):
                    m_tiles[(blk, half)] = mpool.tile(
                        [128, BLKROWS, 128], bf16,
                        tag=f"m{blk}{half}", name=f"m{blk}{half}")

            for blk in range(BLKS):
                for k in range(NK):
                    Gq = gpool.tile([128, BLKROWS, 4 * C], fp8, tag="Gq", name="Gq")
                    for hb in range(2):
                        nc.gpsimd.dma_gather(
                            out_ap=Gq[:, hb * 8:(hb + 1) * 8, :], in_ap=xf[:, :],
                            idxs_ap=I16[:, k, blk, hb * 8:(hb + 1) * 8, :],
                            num_idxs=1024, num_idxs_reg=1024, elem_size=4 * C)
                    s_tiles = [spool.tile([128, BLKROWS, 128], bf16, tag=f"s{h}",
                                          name=f"s{h}") for h in range(2)] \
                        if k > 0 else None
                    # build 16 diag sets for this (k, blk); reused by both halves
                    dts = []
                    for y in range(BLKROWS):
                        col = k * ROWS + blk * BLKROWS + y
                        dt = dpool.tile([128, 4, 128], bf16,
                                        tag=f"d{y}", name=f"d{y}")
                        for corner in range(4):
                            if y in (5, 10) or (y == 2 and corner % 2 == 1):
                                nc.scalar.activation(
                                    out=dt[:, corner, :], in_=idn_sb[:],
                                    func=ACTF.Copy, scale=wcol(corner, col))
                            else:
                                TS(out=dt[:, corner, :], in0=idn_sb[:],
                                   scalar1=wcol(corner, col),
                                   scalar2=None, op0=Alu.mult)
                        dts.append(dt)
                    for half in range(2):
                        for yh in range(2):
                            ps = psum_pool.tile([128, 8, 128], f32, tag="ps",
                                                name="ps")
                            for yy in range(8):
                                y = yh * 8 + yy
                                for corner in range(4):
                                    nc.tensor.matmul(
                                        ps[:, yy, :],
                                        Gq[:, y, corner * C + half * 128:
                                           corner * C + half * 128 + 128],
                                        dts[y][:, corner, :],
                                        start=(corner == 0), stop=(corner == 3))
                            dst = m_tiles[(blk, half)] if k == 0 else s_tiles[half]
                            nc.scalar.activation(
                                out=dst[:, yh * 8:(yh + 1) * 8, :],
                                in_=ps[:], func=ACTF.Copy)
                    if k > 0:
                        for half in range(2):
                            m = m_tiles[(blk, half)]
                            TT(out=m[:], in0=s_tiles[half][:], in1=m[:], op=Alu.max)

                # ---- tail: A-contraction + sigmoid + store ----
                for q in range(4):
                    lg = lg_pool.tile([G4, 512], f32, tag="lg", name="lg")
                    for half in range(2):
                        nc.tensor.matmul(
                            lg[:],
                            a2_sb[:, half * G4:(half + 1) * G4],
                            m_tiles[(blk, half)][:, q * 4:(q + 1) * 4, :],
                            start=(half == 0), stop=(half == 1))
                    att = opool.tile([G4, 512], f32, tag="att", name="att")
                    nc.scalar.activation(out=att, in_=lg[:], func=ACTF.Sigmoid,
                                         bias=c0_sb[:, 0:1])
                    nc.sync.dma_start(
                        out=out[:, blk * BLKROWS + q * 4: blk * BLKROWS + (q + 1) * 4, :],
                        in_=att[:].rearrange("g (y x) -> g y x", y=4))

    _split_waits(nc)
    _lower_libraries(nc)
    return nc


def _marshal(inputs):
    x = np.ascontiguousarray(inputs["x"], dtype=np.float32)
    offset = np.ascontiguousarray(inputs["offset"], dtype=np.float32)
    W0 = np.asarray(inputs["W0"], np.float32); b0 = np.asarray(inputs["b0"], np.float32)
    gamma = np.asarray(inputs["gamma"], np.float32); beta = np.asarray(inputs["beta"], np.float32)
    rm = np.asarray(inputs["run_mean"], np.float32); rv = np.asarray(inputs["run_var"], np.float32)
    W1 = np.asarray(inputs["W1"], np.float32); b1 = np.asarray(inputs["b1"], np.float32)

    inv = gamma / np.sqrt(rv + BN_EPS)
    A = (W1 * inv[None, :]) @ W0              # (4, 256)
    c0 = W1 @ (inv * (b0 - rm) + beta) + b1   # (4,)

    a2 = np.zeros((128, 2 * G4), np.float32)
    a2[:, 0:G4] = A[:, 0:128].T
    a2[:, G4:2 * G4] = A[:, 128:256].T
    a2 = a2.astype(ml_dtypes.bfloat16)
    c0p = c0.reshape(G4, 1).astype(np.float32)
    idn = np.eye(128, dtype=ml_dtypes.bfloat16)

    ky = np.repeat(np.arange(-1, 2), 3).astype(np.float32)   # k//3 - 1
    kx = np.tile(np.arange(-1, 2), 3).astype(np.float32)     # k%3 - 1

    xf_b = []
    for b in range(B):
        # F' = image rows -2..129 zero-padded; table row s holds the 4 corner
        # pixel vectors at positions s, s+1, s+W, s+W+1 of the padded image.
        Ff = np.zeros((132 * W + 2 + 130, C), ml_dtypes.float8_e4m3)
        Ff[2 * W + 1:2 * W + 1 + H * W] = x[b].transpose(1, 2, 0).reshape(H * W, C).astype(ml_dtypes.float8_e4m3)
        T = np.concatenate([Ff[0:NT], Ff[1:NT + 1], Ff[W:NT + W], Ff[W + 1:NT + W + 1]], axis=1)
        xf_b.append(np.ascontiguousarray(T))

    xgrid = np.arange(128, dtype=np.float32)
    in_maps = []
    for core in range(NCORES):
        b = core // 4
        r0 = (core % 4) * ROWS
        # off[x, m=(k,y), c] = offset[b, 2k+c, r0+y, x]
        off = offset[b].reshape(NK, 2, H, W)[:, :, r0:r0 + ROWS, :]
        off_px = off.transpose(3, 0, 2, 1).reshape(128, NC_, 2)
        yv = np.arange(r0, r0 + ROWS, dtype=np.float32)
        ykc = np.broadcast_to((yv[None, :] + ky[:, None]).reshape(1, NC_), (128, NC_))
        xkc = (xgrid[:, None] + np.broadcast_to(
            kx[:, None], (NK, ROWS)).reshape(1, NC_))

        py = off_px[:, :, 0] + ykc                               # [x, (k,y)]
        px = off_px[:, :, 1] + xkc
        y0 = np.floor(py); x0 = np.floor(px)
        fy = (py - y0).astype(np.float32); fx = (px - x0).astype(np.float32)
        v0 = (y0 >= 0) & (y0 <= H - 1); v1 = (y0 + 1 >= 0) & (y0 + 1 <= H - 1)
        u0 = (x0 >= 0) & (x0 <= W - 1); u1 = (x0 + 1 >= 0) & (x0 + 1 <= W - 1)
        wy0 = (1.0 - fy) * v0; wy1 = fy * v1
        wx0 = (1.0 - fx) * u0; wx1 = fx * u1
        wtp = np.concatenate(
            [wy0 * wx0, wy0 * wx1, wy1 * wx0, wy1 * wx1], axis=1
        ).astype(np.float32)                                     # [128, 4*288]

        # gather indices in dma_gather's layout:
        # I16[16g + q, k, blk, y, xh] = sidx[x=16*xh+q, (k, blk*16+y)]
        sidx = (np.clip(y0, -2, 128) * 128 +
                np.clip(x0, -1, 128) + 257).astype(np.int16)
        s5 = sidx.reshape(8, 16, NK, BLKS, BLKROWS)              # [xh, q, k, b, y]
        i16c = np.tile(s5.transpose(1, 2, 3, 4, 0), (8, 1, 1, 1, 1)).copy()
        in_maps.append(dict(xf=xf_b[b], wtp=wtp, i16p=i16c,
                            idn=idn, a2=a2, c0p=c0p))
    return in_maps


def kernel(**inputs):
    if "nc" not in _prog_cache:
        _prog_cache["nc"] = _build_program()
    nc = _prog_cache["nc"]
    in_maps = _marshal(inputs)
    res = run_bass_kernel_spmd(nc, in_maps, list(range(NCORES)))
    out = np.zeros((B, C, H, W), np.float32)
    for core in range(NCORES):
        b = core // 4
        r0 = (core % 4) * ROWS
        att = res.results[core]["out"]               # (4, 32, 128)
        out[b, :, r0:r0 + ROWS, :] = np.tile(att, (C // G4, 1, 1))
    return out
